# revision 4
# baseline (speedup 1.0000x reference)
"""TopK SAE (encode -> top-32 mask -> decode) on 8 trn2 NeuronCores.

Data-parallel over the batch dim N: each core handles N/8 = 512 rows.
W_enc is pre-transposed on the host (the PE contracts over the partition
dim, so the encode needs W_enc^T with D on partitions); W_dec is host-cast
to fp16 for the decode (products accumulate in fp32 in PSUM).

Per-core device pipeline (two batches of 2 row-tiles of 128):
  encode:   raw = x_cent @ W_enc^T via float32r (FP22) matmuls, full PE rate
  stage1:   nc.vector.max top-8 per 512-wide chunk -> 256 candidates/row
            (top-32 of a row provably/empirically lies in these candidates)
  stage2:   4x (max8 + match_replace) over the 256 candidates -> exact
            fp32 threshold t = 32nd largest value per row
  mask:     acts = (raw >= t) * raw, fused scalar_tensor_tensor, in place
  decode:   PE-transpose acts 128x128 blocks (f32r), copy-cast to fp16,
            accumulate recon = acts @ W_dec over 128 f-chunks
Host: shard/transpose/cast prep, gather, loss = mean(sum((recon-x)^2)).
"""

import os
import sys

for _p in ("/opt/trn_rl_repo", "/opt/pypackages"):
    if _p not in sys.path:
        sys.path.insert(0, _p)

import numpy as np

import concourse.bacc as bacc
import concourse.mybir as mybir
from concourse.bass import ds, ts
from concourse.masks import make_identity
from concourse.tile import TileContext
from concourse.bass_utils import run_bass_kernel_spmd

# Problem shape (hardcoded per contract)
N, D, F, K = 4096, 512, 16384, 32
NCORES = 8
NS = N // NCORES          # rows per core = 512
P = 128
NT = NS // P              # 4 row-tiles per core
NBATCH = 2                # row-tiles processed per W_enc stream
FT = 512                  # encode f-tile width (= stage1 chunk width)
NFT = F // FT             # 32
FG = 2                    # f-tiles per W-load group
NCAND = NFT * 8           # 256 candidates per row
FC = 128                  # decode f-chunk (transpose block)
NFC = F // FC             # 128
DECG = 4                  # decode f-chunks per group (one PSUM bank)
MCH = 2048                # mask/acts chunk width
NEG = -3.0e38

f32 = mybir.dt.float32
f32r = mybir.dt.float32r
f16 = mybir.dt.float16


def build(dca: int):
    """Build the per-core Bass program. dca = number of 128-deep contraction
    chunks (4 normally, 5 when b_enc is folded in as an extra chunk)."""
    nc = bacc.Bacc("TRN2", target_bir_lowering=False)

    xT = nc.dram_tensor("xT", [dca * P, NS], f32r, kind="ExternalInput")
    wencT = nc.dram_tensor("wencT", [dca * P, F], f32r, kind="ExternalInput")
    wdec16 = nc.dram_tensor("wdec16", [F, D], f16, kind="ExternalInput")
    acts_d = nc.dram_tensor("acts", [NS, F], f32, kind="ExternalOutput")
    recon_d = nc.dram_tensor("recon", [NS, D], f32, kind="ExternalOutput")

    xT_r = xT.rearrange("(dc p) n -> p dc n", p=P)
    wencT_r = wencT.rearrange("(dc p) f -> p dc f", p=P)
    wdec_r = wdec16.rearrange("(c p) d -> p c d", p=P)

    with TileContext(nc) as tc:
        with (
            tc.tile_pool(name="persist", bufs=1) as persist,
            tc.tile_pool(name="wt", bufs=2) as wtpool,
            tc.tile_pool(name="raw", bufs=2) as rawpool,
            tc.tile_pool(name="cand", bufs=2) as candpool,
            tc.tile_pool(name="work", bufs=2) as workpool,
            tc.tile_pool(name="m8", bufs=8) as m8pool,
            tc.tile_pool(name="at", bufs=3) as atpool,
            tc.tile_pool(name="wd", bufs=3) as wdpool,
            tc.tile_pool(name="rc", bufs=2) as rcpool,
            tc.tile_pool(name="encps", bufs=4, space="PSUM") as encps,
            tc.tile_pool(name="trps", bufs=2, space="PSUM") as trps,
            tc.tile_pool(name="decps", bufs=2, space="PSUM") as decps,
        ):
            ident = persist.tile([P, P], f32)
            make_identity(nc, ident)

            xt_sb = persist.tile([P, dca, NS], f32r)
            nc.sync.dma_start(xt_sb[:], xT_r[:])

            for b in range(NT // NBATCH):
                raws = []
                cands = []
                for nt2 in range(NBATCH):
                    raws.append(rawpool.tile([P, F], f32, tag="raw", name=f"raw{nt2}"))
                    cands.append(candpool.tile([P, NCAND], f32, tag="cand", name=f"cand{nt2}"))

                # ---- encode + stage1 candidates ----
                for fg in range(NFT // FG):
                    wt = wtpool.tile([P, dca, FG * FT], f32r, tag="wt")
                    nc.sync.dma_start(
                        wt[:], wencT_r[:, :, ds(fg * FG * FT, FG * FT)]
                    )
                    for nt2 in range(NBATCH):
                        nt = b * NBATCH + nt2
                        pss = [
                            encps.tile([P, FT], f32, tag="encps", name=f"encps{i}")
                            for i in range(FG)
                        ]
                        for dc in range(dca):
                            for ft in range(FG):
                                nc.tensor.matmul(
                                    pss[ft],
                                    lhsT=xt_sb[:, dc, ds(nt * P, P)],
                                    rhs=wt[:, dc, ds(ft * FT, FT)],
                                    start=(dc == 0),
                                    stop=(dc == dca - 1),
                                )
                        for ft in range(FG):
                            fa = fg * FG + ft
                            nc.scalar.copy(
                                raws[nt2][:, ds(fa * FT, FT)], pss[ft]
                            )
                            nc.vector.max(
                                out=cands[nt2][:, ds(fa * 8, 8)],
                                in_=raws[nt2][:, ds(fa * FT, FT)],
                            )

                # ---- stage2: exact top-K threshold from candidates ----
                t_aps = []
                for nt2 in range(NBATCH):
                    work = workpool.tile([P, NCAND], f32, tag="work")
                    nc.vector.tensor_copy(work[:], cands[nt2][:])
                    niter = (K + 7) // 8
                    m8s = [
                        m8pool.tile([P, 8], f32, tag=f"m8_{j}", name=f"m8_{j}")
                        for j in range(niter)
                    ]
                    for j in range(niter):
                        nc.vector.max(out=m8s[j][:], in_=work[:])
                        if j < niter - 1:
                            nc.vector.match_replace(
                                out=work[:],
                                in_to_replace=m8s[j][:],
                                in_values=work[:],
                                imm_value=NEG,
                            )
                    t_aps.append(m8s[niter - 1][:, ds((K - 1) % 8, 1)])

                # ---- mask (acts = (raw>=t)*raw in place) + acts DMA out ----
                for nt2 in range(NBATCH):
                    nt = b * NBATCH + nt2
                    for ch in range(F // MCH):
                        sl = ds(ch * MCH, MCH)
                        nc.vector.scalar_tensor_tensor(
                            out=raws[nt2][:, sl],
                            in0=raws[nt2][:, sl],
                            scalar=t_aps[nt2],
                            in1=raws[nt2][:, sl],
                            op0=mybir.AluOpType.is_ge,
                            op1=mybir.AluOpType.mult,
                        )
                        nc.sync.dma_start(
                            acts_d[ds(nt * P, P), sl], raws[nt2][:, sl]
                        )

                # ---- decode: recon += acts @ W_dec over f-chunks ----
                dec_ps = [
                    decps.tile([P, D], f32, tag="decps", name=f"decps{i}")
                    for i in range(NBATCH)
                ]
                for g in range(NFC // DECG):
                    wd = wdpool.tile([P, DECG, D], f16, tag="wd")
                    nc.sync.dma_start(wd[:], wdec_r[:, ds(g * DECG, DECG), :])
                    for nt2 in range(NBATCH):
                        tr = trps.tile([P, DECG, P], f32, tag="trps")
                        for j in range(DECG):
                            nc.tensor.transpose(
                                tr[:, j, :],
                                raws[nt2][:, ds((g * DECG + j) * P, P)],
                                ident[:],
                            )
                        at = atpool.tile([P, DECG, P], f16, tag="at")
                        nc.scalar.copy(at[:], tr[:])
                        for j in range(DECG):
                            nc.tensor.matmul(
                                dec_ps[nt2],
                                lhsT=at[:, j, :],
                                rhs=wd[:, j, :],
                                start=(g == 0 and j == 0),
                                stop=(g == NFC // DECG - 1 and j == DECG - 1),
                            )
                for nt2 in range(NBATCH):
                    nt = b * NBATCH + nt2
                    rc = rcpool.tile([P, D], f32, tag="rc")
                    nc.scalar.copy(rc[:], dec_ps[nt2])
                    nc.sync.dma_start(recon_d[ds(nt * P, P), :], rc[:])

    nc.compile()
    return nc


_cache = {}


def _get_nc(dca: int):
    if dca not in _cache:
        _cache[dca] = build(dca)
    return _cache[dca]


def run(inputs, trace=False, trace_cores=None):
    x = np.asarray(inputs["x"], dtype=np.float32)
    W_enc = np.asarray(inputs["W_enc"], dtype=np.float32)
    W_dec = np.asarray(inputs["W_dec"], dtype=np.float32)
    b_enc = np.asarray(inputs["b_enc"], dtype=np.float32)
    b_dec = np.asarray(inputs["b_dec"], dtype=np.float32)
    k = int(np.asarray(inputs["num_winners"]))
    assert k == K, f"kernel specialized for K={K}, got {k}"
    assert x.shape == (N, D) and W_enc.shape == (F, D)

    x_cent = x - b_dec[None, :]

    has_benc = bool(np.any(b_enc))
    dca = D // P + (1 if has_benc else 0)

    # host-side weight prep (layout for the PE): W_enc^T with D on
    # partitions; optional extra contraction chunk folds b_enc in via an
    # all-ones row of x.
    wencT = np.ascontiguousarray(W_enc.T)          # [D, F]
    if has_benc:
        pad = np.zeros((P, F), np.float32)
        pad[0, :] = b_enc
        wencT = np.concatenate([wencT, pad], axis=0)
    wdec16 = W_dec.astype(np.float16)              # [F, D]

    nc = _get_nc(dca)

    in_maps = []
    for c in range(NCORES):
        xs = x_cent[c * NS : (c + 1) * NS]          # [NS, D]
        xsT = np.ascontiguousarray(xs.T)            # [D, NS]
        if has_benc:
            pad = np.zeros((P, NS), np.float32)
            pad[0, :] = 1.0
            xsT = np.concatenate([xsT, pad], axis=0)
        in_maps.append({"xT": xsT, "wencT": wencT, "wdec16": wdec16})

    res = run_bass_kernel_spmd(
        nc,
        in_maps,
        core_ids=list(range(NCORES)),
        trace=trace,
        trace_cores=trace_cores,
    )

    acts = np.concatenate([res.results[c]["acts"] for c in range(NCORES)], axis=0)
    recon = np.concatenate([res.results[c]["recon"] for c in range(NCORES)], axis=0)
    recon = recon + b_dec[None, :]

    diff = recon.astype(np.float32) - x
    loss = np.float32(np.mean(np.sum(diff * diff, axis=-1, dtype=np.float32)))
    return (loss, recon, acts), res


def kernel(**inputs):
    out, _ = run(inputs, trace=False)
    return out


# revision 6
# speedup vs baseline: 1.0710x; 1.0710x over previous
"""TopK SAE (encode -> top-32 mask -> decode) on 8 trn2 NeuronCores.

Data-parallel over the batch dim N: each core handles N/8 = 512 rows.
W_enc is pre-transposed on the host (the PE contracts over the partition
dim, so the encode needs W_enc^T with D on partitions); W_dec is host-cast
to fp16 for the decode (products accumulate in fp32 in PSUM).

Per-core device pipeline (two batches of 2 row-tiles of 128):
  encode:   raw = x_cent @ W_enc^T via float32r (FP22) matmuls, full PE rate
  stage1:   nc.vector.max top-8 per 512-wide chunk -> 256 candidates/row
            (top-32 of a row provably/empirically lies in these candidates)
  stage2:   4x (max8 + match_replace) over the 256 candidates -> exact
            fp32 threshold t = 32nd largest value per row
  mask:     acts = (raw >= t) * raw, fused scalar_tensor_tensor, in place
  decode:   PE-transpose acts 128x128 blocks (f32r), copy-cast to fp16,
            accumulate recon = acts @ W_dec over 128 f-chunks
Host: shard/transpose/cast prep, gather, loss = mean(sum((recon-x)^2)).
"""

import os
import sys

for _p in ("/opt/trn_rl_repo", "/opt/pypackages"):
    if _p not in sys.path:
        sys.path.insert(0, _p)

import numpy as np

import concourse.bacc as bacc
import concourse.mybir as mybir
from concourse.bass import ds, ts
from concourse.masks import make_identity
from concourse.tile import TileContext
from concourse.bass_utils import run_bass_kernel_spmd

# Problem shape (hardcoded per contract)
N, D, F, K = 4096, 512, 16384, 32
NCORES = 8
NS = N // NCORES          # rows per core = 512
P = 128
NT = NS // P              # 4 row-tiles per core
NBATCH = 2                # row-tiles processed per W_enc stream
FT = 512                  # encode f-tile width (= stage1 chunk width)
NFT = F // FT             # 32
FG = 2                    # f-tiles per W-load group
NCAND = NFT * 8           # 256 candidates per row
FC = 128                  # decode f-chunk (transpose block)
NFC = F // FC             # 128
DECG = 4                  # decode f-chunks per group (one PSUM bank)
MCH = 2048                # mask/acts chunk width
NEG = -3.0e38

f32 = mybir.dt.float32
f32r = mybir.dt.float32r
f16 = mybir.dt.float16
u8 = mybir.dt.uint8


def build(dca: int):
    """Build the per-core Bass program. dca = number of 128-deep contraction
    chunks (4 normally, 5 when b_enc is folded in as an extra chunk)."""
    nc = bacc.Bacc("TRN2", target_bir_lowering=False)

    xT = nc.dram_tensor("xT", [dca * P, NS], f32r, kind="ExternalInput")
    wencT = nc.dram_tensor("wencT", [dca * P, F], f32r, kind="ExternalInput")
    wdec16 = nc.dram_tensor("wdec16", [F, D], f16, kind="ExternalInput")
    acts_d = nc.dram_tensor("acts", [NS, F], f32, kind="ExternalOutput")
    recon_d = nc.dram_tensor("recon", [NS, D], f32, kind="ExternalOutput")
    mask48_d = nc.dram_tensor("mask48", [NS, F], u8, kind="ExternalOutput")

    xT_r = xT.rearrange("(dc p) n -> p dc n", p=P)
    wencT_r = wencT.rearrange("(dc p) f -> p dc f", p=P)
    wdec_r = wdec16.rearrange("(c p) d -> p c d", p=P)

    with TileContext(nc) as tc:
        with (
            tc.tile_pool(name="persist", bufs=1) as persist,
            tc.tile_pool(name="wt", bufs=2) as wtpool,
            tc.tile_pool(name="raw", bufs=2) as rawpool,
            tc.tile_pool(name="cand", bufs=2) as candpool,
            tc.tile_pool(name="work", bufs=2) as workpool,
            tc.tile_pool(name="m8", bufs=8) as m8pool,
            tc.tile_pool(name="at", bufs=3) as atpool,
            tc.tile_pool(name="wd", bufs=3) as wdpool,
            tc.tile_pool(name="rc", bufs=2) as rcpool,
            tc.tile_pool(name="m48", bufs=3) as m48pool,
            tc.tile_pool(name="encps", bufs=4, space="PSUM") as encps,
            tc.tile_pool(name="trps", bufs=2, space="PSUM") as trps,
            tc.tile_pool(name="decps", bufs=2, space="PSUM") as decps,
        ):
            ident = persist.tile([P, P], f32)
            make_identity(nc, ident)

            xt_sb = persist.tile([P, dca, NS], f32r)
            nc.sync.dma_start(xt_sb[:], xT_r[:])

            for b in range(NT // NBATCH):
                raws = []
                cands = []
                for nt2 in range(NBATCH):
                    raws.append(rawpool.tile([P, F], f32, tag="raw", name=f"raw{nt2}"))
                    cands.append(candpool.tile([P, NCAND], f32, tag="cand", name=f"cand{nt2}"))

                # ---- encode + stage1 candidates ----
                for fg in range(NFT // FG):
                    wt = wtpool.tile([P, dca, FG * FT], f32r, tag="wt")
                    for dc in range(dca):
                        nc.sync.dma_start(
                            wt[:, dc, :],
                            wencT_r[:, dc, ds(fg * FG * FT, FG * FT)],
                        )
                    for nt2 in range(NBATCH):
                        nt = b * NBATCH + nt2
                        pss = [
                            encps.tile([P, FT], f32, tag="encps", name=f"encps{i}")
                            for i in range(FG)
                        ]
                        for dc in range(dca):
                            for ft in range(FG):
                                nc.tensor.matmul(
                                    pss[ft],
                                    lhsT=xt_sb[:, dc, ds(nt * P, P)],
                                    rhs=wt[:, dc, ds(ft * FT, FT)],
                                    start=(dc == 0),
                                    stop=(dc == dca - 1),
                                )
                        for ft in range(FG):
                            fa = fg * FG + ft
                            nc.scalar.copy(
                                raws[nt2][:, ds(fa * FT, FT)], pss[ft]
                            )
                            nc.vector.max(
                                out=cands[nt2][:, ds(fa * 8, 8)],
                                in_=raws[nt2][:, ds(fa * FT, FT)],
                            )

                # ---- stage2: exact top-K threshold from candidates ----
                # runs 6 max8 rounds: rank K=32 gives the acts threshold,
                # rank 48 gives a slack threshold whose u8 mask lets the
                # host re-rank the boundary candidates exactly.
                t_aps = []
                t48_aps = []
                for nt2 in range(NBATCH):
                    work = workpool.tile([P, NCAND], f32, tag="work")
                    nc.vector.tensor_copy(work[:], cands[nt2][:])
                    niter = 6
                    m8s = [
                        m8pool.tile([P, 8], f32, tag=f"m8_{j}", name=f"m8_{j}")
                        for j in range(niter)
                    ]
                    for j in range(niter):
                        nc.vector.max(out=m8s[j][:], in_=work[:])
                        if j < niter - 1:
                            nc.vector.match_replace(
                                out=work[:],
                                in_to_replace=m8s[j][:],
                                in_values=work[:],
                                imm_value=NEG,
                            )
                    t_aps.append(m8s[(K - 1) // 8][:, ds((K - 1) % 8, 1)])
                    t48_aps.append(m8s[niter - 1][:, ds(7, 1)])

                # ---- mask (acts = (raw>=t)*raw in place) + acts DMA out ----
                for nt2 in range(NBATCH):
                    nt = b * NBATCH + nt2
                    for ch in range(F // MCH):
                        sl = ds(ch * MCH, MCH)
                        m48 = m48pool.tile([P, MCH], u8, tag="m48")
                        nc.vector.tensor_scalar(
                            m48[:],
                            raws[nt2][:, sl],
                            t48_aps[nt2],
                            None,
                            op0=mybir.AluOpType.is_ge,
                        )
                        nc.sync.dma_start(
                            mask48_d[ds(nt * P, P), sl], m48[:]
                        )
                        nc.vector.scalar_tensor_tensor(
                            out=raws[nt2][:, sl],
                            in0=raws[nt2][:, sl],
                            scalar=t_aps[nt2],
                            in1=raws[nt2][:, sl],
                            op0=mybir.AluOpType.is_ge,
                            op1=mybir.AluOpType.mult,
                        )
                        nc.sync.dma_start(
                            acts_d[ds(nt * P, P), sl], raws[nt2][:, sl]
                        )

                # ---- decode: recon += acts @ W_dec over f-chunks ----
                dec_ps = [
                    decps.tile([P, D], f32, tag="decps", name=f"decps{i}")
                    for i in range(NBATCH)
                ]
                for g in range(NFC // DECG):
                    wd = wdpool.tile([P, DECG, D], f16, tag="wd")
                    nc.sync.dma_start(wd[:], wdec_r[:, ds(g * DECG, DECG), :])
                    for nt2 in range(NBATCH):
                        tr = trps.tile([P, DECG, P], f32, tag="trps")
                        for j in range(DECG):
                            nc.tensor.transpose(
                                tr[:, j, :],
                                raws[nt2][:, ds((g * DECG + j) * P, P)],
                                ident[:],
                            )
                        at = atpool.tile([P, DECG, P], f16, tag="at")
                        nc.scalar.copy(at[:], tr[:])
                        for j in range(DECG):
                            nc.tensor.matmul(
                                dec_ps[nt2],
                                lhsT=at[:, j, :],
                                rhs=wd[:, j, :],
                                start=(g == 0 and j == 0),
                                stop=(g == NFC // DECG - 1 and j == DECG - 1),
                            )
                for nt2 in range(NBATCH):
                    nt = b * NBATCH + nt2
                    rc = rcpool.tile([P, D], f32, tag="rc")
                    nc.scalar.copy(rc[:], dec_ps[nt2])
                    nc.sync.dma_start(recon_d[ds(nt * P, P), :], rc[:])

    nc.compile()
    return nc


_cache = {}


def _get_nc(dca: int):
    if dca not in _cache:
        _cache[dca] = build(dca)
    return _cache[dca]


def run(inputs, trace=False, trace_cores=None):
    x = np.asarray(inputs["x"], dtype=np.float32)
    W_enc = np.asarray(inputs["W_enc"], dtype=np.float32)
    W_dec = np.asarray(inputs["W_dec"], dtype=np.float32)
    b_enc = np.asarray(inputs["b_enc"], dtype=np.float32)
    b_dec = np.asarray(inputs["b_dec"], dtype=np.float32)
    k = int(np.asarray(inputs["num_winners"]))
    assert k == K, f"kernel specialized for K={K}, got {k}"
    assert x.shape == (N, D) and W_enc.shape == (F, D)

    x_cent = x - b_dec[None, :]

    has_benc = bool(np.any(b_enc))
    dca = D // P + (1 if has_benc else 0)

    # host-side weight prep (layout for the PE): W_enc^T with D on
    # partitions; optional extra contraction chunk folds b_enc in via an
    # all-ones row of x.
    wencT = np.ascontiguousarray(W_enc.T)          # [D, F]
    if has_benc:
        pad = np.zeros((P, F), np.float32)
        pad[0, :] = b_enc
        wencT = np.concatenate([wencT, pad], axis=0)
    wdec16 = W_dec.astype(np.float16)              # [F, D]

    nc = _get_nc(dca)

    in_maps = []
    for c in range(NCORES):
        xs = x_cent[c * NS : (c + 1) * NS]          # [NS, D]
        xsT = np.ascontiguousarray(xs.T)            # [D, NS]
        if has_benc:
            pad = np.zeros((P, NS), np.float32)
            pad[0, :] = 1.0
            xsT = np.concatenate([xsT, pad], axis=0)
        in_maps.append({"xT": xsT, "wencT": wencT, "wdec16": wdec16})

    res = run_bass_kernel_spmd(
        nc,
        in_maps,
        core_ids=list(range(NCORES)),
        trace=trace,
        trace_cores=trace_cores,
    )

    acts = np.concatenate([res.results[c]["acts"] for c in range(NCORES)], axis=0)
    recon = np.concatenate([res.results[c]["recon"] for c in range(NCORES)], axis=0)
    mask48 = np.concatenate(
        [res.results[c]["mask48"] for c in range(NCORES)], axis=0
    )
    recon = recon + b_dec[None, :]

    # Exact boundary re-ranking. The device encode runs at FP22 (float32r),
    # which can swap winners whose fp32 gap is below ~1e-4 (a few dozen rows
    # out of 4096). mask48 marks each row's top-48 device values; recompute
    # those dot products exactly in fp32 and patch the handful of rows whose
    # top-32 set differs.
    rows, cols = np.nonzero(mask48)
    # per-row segments (rows from np.nonzero are sorted)
    row_starts = np.searchsorted(rows, np.arange(N + 1))
    exact = np.einsum(
        "nd,nd->n", x_cent[rows], W_enc[cols], dtype=np.float32
    ) + b_enc[cols]
    for n in range(N):
        s, e = row_starts[n], row_starts[n + 1]
        cs = cols[s:e]
        ev = exact[s:e]
        if len(cs) < K:
            continue
        order = np.argsort(-ev, kind="stable")
        true_set = cs[order[:K]]
        sel = cs[acts[n, cs] != 0]
        if len(sel) == K and np.array_equal(np.sort(true_set), np.sort(sel)):
            continue
        acts[n, sel] = 0.0
        acts[n, true_set] = ev[order[:K]]
        recon[n] = acts[n, true_set] @ W_dec[true_set] + b_dec

    diff = recon.astype(np.float32) - x
    loss = np.float32(np.mean(np.sum(diff * diff, axis=-1, dtype=np.float32)))
    return (loss, recon, acts), res


def kernel(**inputs):
    out, _ = run(inputs, trace=False)
    return out
